# revision 2
# baseline (speedup 1.0000x reference)
"""Trainium2 Bass kernel for nn_MetaPosterior — v5: single APGather route.

The v4 hybrid (PE one-hot gathers for 2048 pairs + Pool ap_gather for 2048)
spent ~20 us/iter of combined Tensor+Scalar+Vector work on the PE route,
while the trace showed the Pool-route APGather handling 512 idx/core in
394 ns.  v5 routes ALL 4096 pairs per core through one APGather
(1024 idx per gpsimd core block) and does the pair math in 3 DVE ops:

  table rows t = k*1025 + token store (q, h') f16 with
    q  = -0.5 * g
    h' = h - 0.5 * t_k          (h = g * (theta - m_k), t_k = sum_t h)
  per pair (slot indices t0, t1):
    sums = gather[t0] + gather[t1]      -> (qs, u) interleaved, one TT add
    w    = u * u                        (strided TT mult)
    fin  = (1 + qs) * w                 (STT, accum -> red[:,0])
  lp_pair = fin + a'[t0] + a'[t1] - LOG2PI, with the a' term folded into
  per-row counts (cnt * asp accumulated in red[:,2], as in v4).

Per iteration: 1 APGather (GpSimd) + 4 DVE ops; everything else idle.
Gather output is triple-buffered so gather i+1 overlaps DVE iter i.
"""

import numpy as np

import concourse.bacc as bacc
import concourse.mybir as mybir
from concourse.bass_utils import run_bass_kernel_spmd

LOG2PI = float(np.log(2.0 * np.pi))
DIM, K, P, M_COND = 1024, 8, 4, 2
D1 = DIM + 1                      # 1025
N_CORES = 8
A_PER_CORE = DIM // N_CORES       # 128
TRIPLES = A_PER_CORE * P * K      # 4096 pairs per core
TBL_ROWS = K * D1                 # 8200 table rows
DF = 2                            # f16 fields per row: [q, h']
CNT_COLS = (TBL_ROWS + 127) // 128  # 65

N_BLOCKS = 8                      # gpsimd core blocks (16 partitions each)
PAIRS_B = TRIPLES // N_BLOCKS     # 512 pairs per block
NIDX = 2 * PAIRS_B                # 1024 gather idx per block
IDX_COLS = NIDX // 16             # 64
RED_COLS = 16

_PROGS = {}


def _build_program(iters=1):
    f32 = mybir.dt.float32
    f16 = mybir.dt.float16
    i16 = mybir.dt.int16
    alu = mybir.AluOpType
    nc = bacc.Bacc("TRN2", detect_race_conditions=False)

    tbl = nc.dram_tensor("tbl", [128, TBL_ROWS, DF], f16, kind="ExternalInput")
    idx = nc.dram_tensor("idx", [128, IDX_COLS], i16, kind="ExternalInput")
    cnt = nc.dram_tensor("cnt", [128, CNT_COLS], f32, kind="ExternalInput")
    asp = nc.dram_tensor("asp", [128, CNT_COLS], f32, kind="ExternalInput")
    out = nc.dram_tensor("out", [128, RED_COLS], f32, kind="ExternalOutput")

    import contextlib
    with contextlib.ExitStack() as _es:
        ec = _es.enter_context
        tbl_sb = ec(nc.sbuf_tensor("tbl_sb", [128, TBL_ROWS, DF], f16))
        idx_sb = ec(nc.sbuf_tensor("idx_sb", [128, IDX_COLS], i16))
        cnt_sb = ec(nc.sbuf_tensor("cnt_sb", [128, CNT_COLS], f32))
        asp_sb = ec(nc.sbuf_tensor("asp_sb", [128, CNT_COLS], f32))
        gath = [ec(nc.sbuf_tensor(f"gath{b}", [128, NIDX, DF], f16))
                for b in range(3)]
        sums = ec(nc.sbuf_tensor("sums", [128, PAIRS_B, DF], f16))
        w = ec(nc.sbuf_tensor("w", [128, PAIRS_B], f16))
        junk = ec(nc.sbuf_tensor("junk", [128, PAIRS_B], f16))
        j2 = ec(nc.sbuf_tensor("j2", [128, CNT_COLS], f32))
        red = ec(nc.sbuf_tensor("red", [128, RED_COLS], f32))
        s_in = ec(nc.semaphore("s_in"))
        s_g = ec(nc.semaphore("s_g"))
        s_v = ec(nc.semaphore("s_v"))
        s_o = ec(nc.semaphore("s_o"))
        block = ec(nc.Block())

        @block.gpsimd
        def _(gp):
            gp.dma_start(idx_sb[:], idx[:]).then_inc(s_in, 16)
            gp.dma_start(tbl_sb[:], tbl[:]).then_inc(s_in, 16)
            gp.dma_start(cnt_sb[:], cnt[:]).then_inc(s_in, 16)
            gp.dma_start(asp_sb[:], asp[:]).then_inc(s_in, 16)
            gp.wait_ge(s_in, 64)
            for i in range(iters):
                if i >= 3:
                    gp.wait_ge(s_v, i - 2)
                gp.ap_gather(
                    gath[i % 3][:], tbl_sb[:], idx_sb[:],
                    channels=128, num_elems=TBL_ROWS, d=DF, num_idxs=NIDX,
                ).then_inc(s_g, 1)

        @block.vector
        def _(v):
            v.memset(red[:], 0.0)
            v.wait_ge(s_in, 64)
            for i in range(iters):
                g = gath[i % 3]
                v.scalar_tensor_tensor(
                    j2[:], cnt_sb[:], 1.0, asp_sb[:], alu.mult, alu.mult,
                    accum_out=red[:, 2:3],
                )
                v.wait_ge(s_g, i + 1)
                v.tensor_tensor(sums[:], g[:, 0:PAIRS_B, :],
                                g[:, PAIRS_B:NIDX, :], alu.add)
                v.tensor_tensor(w[:], sums[:, :, 1], sums[:, :, 1], alu.mult)
                v.scalar_tensor_tensor(
                    junk[:], sums[:, :, 0], 1.0, w[:], alu.add, alu.mult,
                    accum_out=red[:, 0:1],
                ).then_inc(s_v, 1)

        @block.sync
        def _(s):
            s.wait_ge(s_v, iters)
            s.dma_start(out[:], red[:]).then_inc(s_o, 16)
            s.wait_ge(s_o, 16)

    nc.finalize()
    return nc


def _get_program(iters=1):
    if iters not in _PROGS:
        _PROGS[iters] = _build_program(iters)
    return _PROGS[iters]


def _make_tables(meta_theta, m_ks, grads_v):
    g = np.asarray(grads_v, np.float64)
    c = np.asarray(meta_theta, np.float64)[None, :] - np.asarray(m_ks, np.float64)
    h = g * c
    t_k = h.sum(axis=1)
    a_f = 0.5 * np.log(g) - 0.5 * g * c * c
    hp = h - 0.5 * t_k[:, None]
    ap = a_f + t_k[:, None] * h - 0.5 * (t_k * t_k)[:, None]

    tbl = np.empty((TBL_ROWS, DF), np.float16)
    tbl[:, 0] = (-0.5 * g).ravel().astype(np.float16)
    tbl[:, 1] = hp.ravel().astype(np.float16)

    asp = np.zeros(128 * CNT_COLS, np.float64)
    asp[:TBL_ROWS] = ap.ravel()
    asp = np.ascontiguousarray(asp.reshape(CNT_COLS, 128).T.astype(np.float32))
    return tbl, asp


def _device_inputs(meta_theta, m_ks, grads_v, perms):
    tbl, asp = _make_tables(meta_theta, m_ks, grads_v)
    tbl_rep = np.ascontiguousarray(np.broadcast_to(tbl[None], (128, TBL_ROWS, DF)))

    perms01 = np.ascontiguousarray(np.asarray(perms)[:, :, :, :2])
    kvec = np.tile(np.arange(K, dtype=np.int64), TRIPLES // K)

    in_maps = []
    for core in range(N_CORES):
        sl = perms01[core * A_PER_CORE : (core + 1) * A_PER_CORE]
        sl = sl.reshape(TRIPLES, 2).astype(np.int64)
        comb = kvec[:, None] * D1 + sl                    # (4096, 2)

        cntv = np.bincount(comb.ravel(), minlength=128 * CNT_COLS)
        cnt = np.ascontiguousarray(
            cntv.reshape(CNT_COLS, 128).T.astype(np.float32)
        )

        # per block b: idx list = slot0 of its 512 pairs, then slot1
        cb = comb.reshape(N_BLOCKS, PAIRS_B, 2)
        L = np.empty((N_BLOCKS, NIDX), np.int16)
        L[:, :PAIRS_B] = cb[:, :, 0]
        L[:, PAIRS_B:] = cb[:, :, 1]
        # wrapped layout: position n of block b -> idx128[16b + n%16, n//16]
        idx128 = L.reshape(N_BLOCKS, IDX_COLS, 16).transpose(0, 2, 1)
        idx128 = np.ascontiguousarray(idx128.reshape(128, IDX_COLS))

        in_maps.append({
            "tbl": tbl_rep, "idx": idx128, "cnt": cnt, "asp": asp,
        })
    return in_maps


def _finalize(partials, meta_theta, alpha):
    p = np.asarray(partials, np.float64)
    total = float(p[:, ::16, 0].sum() + p[:, :, 2].sum())
    sum_lp = total - LOG2PI * (N_CORES * TRIPLES)
    loss_pred = sum_lp / (P * M_COND * K)
    mt = np.asarray(meta_theta, np.float64)
    a = float(alpha)
    lp_prior = -0.5 * (D1 * LOG2PI + D1 * np.log(a) + float(mt @ mt) / a)
    loss = (1.0 - 1.0 / K) * lp_prior + loss_pred
    return np.float32(-loss)


def run_device(in_maps, iters=1, **kwargs):
    nc = _get_program(iters)
    return run_bass_kernel_spmd(nc, in_maps, list(range(N_CORES)), **kwargs)


def kernel(meta_theta, m_ks, grads_v, perms, alpha):
    in_maps = _device_inputs(meta_theta, m_ks, grads_v, perms)
    last_err = None
    for _ in range(3):
        try:
            res = run_device(in_maps)
            break
        except Exception as e:  # noqa: BLE001
            last_err = e
    else:
        raise last_err
    partials = np.stack([r["out"] for r in res.results])
    return _finalize(partials, meta_theta, alpha)


# revision 3
# speedup vs baseline: 2.5111x; 2.5111x over previous
"""Trainium2 Bass kernel for nn_MetaPosterior — v7: hybrid PE + small APGather.

Measured ap_gather truth (hidden sequencer stall, d/num_elems-independent):
cost = ~28 ns x num_idxs per Q7 core block.  So a full-gather route costs
28*1024 = 28.7 us/iter (v5 measured 34), and the gather is only worth it
for a slice of the pairs.  v7 splits:

- Gather route: G=1024 pairs (256 idx per block, ~7.2 us hidden GPSIMD,
  overlapped) incl. all pairs touching table rows >= 8192 (fringe).
  DVE: fused (q,h) add -> w = u^2 -> (1+qs)*w accum.  q = -0.5*g.
- PE route: 3072 pairs in 6 chunks of 512.  Per chunk:
    Y0 = SBO^T x oh0, Y1 = SBO^T x oh1   (SBO: cols j<64 = q-block j,
                                          cols 64+j = h'-block j)
    ACT copies Y0,Y1 -> bf16 SBUF
    DVE: masked_s = Yc_s * m_s  (m has TWO hot rows per column: q0 and
         64+q0 -> extracts q-val and h-val in one op), msum = m0+m1
    PE: FINq += S_q_g^T x msum, FINu += S_u_g^T x msum  (S_*_g: 16-col
        band per chunk, rows <64 / >=64) -> FINq/FINu rows 16g..16g+16
        hold qs / u replicated x16.
  Finals once per iter: w = FINu*FINu, fin = (1+FINq)*w accum red[:,4].
- j2 = cnt*asp accum red[:,2] covers the a' term for ALL pairs.

GPSIMD issues gathers with NO waits: gathered content is iteration-
invariant, so benign overwrite; avoids the ~30us blocked-wait stall.
"""

import numpy as np
import ml_dtypes

import concourse.bacc as bacc
import concourse.mybir as mybir
from concourse.bass_utils import run_bass_kernel_spmd

LOG2PI = float(np.log(2.0 * np.pi))
DIM, K, P, M_COND = 1024, 8, 4, 2
D1 = DIM + 1
N_CORES = 8
A_PER_CORE = DIM // N_CORES
TRIPLES = A_PER_CORE * P * K      # 4096 pairs per core
TBL_ROWS = K * D1                 # 8200
DF = 2
CNT_COLS = (TBL_ROWS + 127) // 128  # 65

G_PAIRS = 1024                    # gather-route pairs per core
GP_B = G_PAIRS // 8               # 128 pairs per block
NIDX = 2 * GP_B                   # 256 idx per block
IDX_COLS = NIDX // 16             # 16

N_CH = 6                          # PE chunks
CH = 512                          # pairs per chunk
PE_PAIRS = N_CH * CH              # 3072
RED_COLS = 16

_PROGS = {}


def _build_program(iters=1):
    f32 = mybir.dt.float32
    f16 = mybir.dt.float16
    bf16 = mybir.dt.bfloat16
    i16 = mybir.dt.int16
    alu = mybir.AluOpType
    nc = bacc.Bacc("TRN2", detect_race_conditions=False)

    tbl = nc.dram_tensor("tbl", [128, TBL_ROWS, DF], f16, kind="ExternalInput")
    idx = nc.dram_tensor("idx", [128, IDX_COLS], i16, kind="ExternalInput")
    cnt = nc.dram_tensor("cnt", [128, CNT_COLS], f32, kind="ExternalInput")
    asp = nc.dram_tensor("asp", [128, CNT_COLS], f32, kind="ExternalInput")
    oh0 = nc.dram_tensor("oh0", [128, PE_PAIRS], bf16, kind="ExternalInput")
    oh1 = nc.dram_tensor("oh1", [128, PE_PAIRS], bf16, kind="ExternalInput")
    m0 = nc.dram_tensor("m0", [128, PE_PAIRS], bf16, kind="ExternalInput")
    m1 = nc.dram_tensor("m1", [128, PE_PAIRS], bf16, kind="ExternalInput")
    sbo = nc.dram_tensor("sbo", [128, 128], bf16, kind="ExternalInput")
    stat = nc.dram_tensor("stat", [128, 2 * N_CH * 128], bf16,
                          kind="ExternalInput")
    out = nc.dram_tensor("out", [128, RED_COLS], f32, kind="ExternalOutput")

    import contextlib
    with contextlib.ExitStack() as _es:
        ec = _es.enter_context
        tbl_sb = ec(nc.sbuf_tensor("tbl_sb", [128, TBL_ROWS, DF], f16))
        idx_sb = ec(nc.sbuf_tensor("idx_sb", [128, IDX_COLS], i16))
        cnt_sb = ec(nc.sbuf_tensor("cnt_sb", [128, CNT_COLS], f32))
        asp_sb = ec(nc.sbuf_tensor("asp_sb", [128, CNT_COLS], f32))
        oh0_sb = ec(nc.sbuf_tensor("oh0_sb", [128, PE_PAIRS], bf16))
        oh1_sb = ec(nc.sbuf_tensor("oh1_sb", [128, PE_PAIRS], bf16))
        m0_sb = ec(nc.sbuf_tensor("m0_sb", [128, PE_PAIRS], bf16))
        m1_sb = ec(nc.sbuf_tensor("m1_sb", [128, PE_PAIRS], bf16))
        sbo_sb = ec(nc.sbuf_tensor("sbo_sb", [128, 128], bf16))
        stat_sb = ec(nc.sbuf_tensor("stat_sb", [128, 2 * N_CH * 128], bf16))
        gath = [ec(nc.sbuf_tensor(f"gath{b}", [128, NIDX, DF], f16))
                for b in range(3)]
        sums = ec(nc.sbuf_tensor("sums", [128, GP_B, DF], f16))
        w2 = ec(nc.sbuf_tensor("w2", [128, GP_B], f16))
        junk = ec(nc.sbuf_tensor("junk", [128, GP_B], f16))
        y0c = [ec(nc.sbuf_tensor(f"y0c{b}", [128, CH], bf16)) for b in range(2)]
        y1c = [ec(nc.sbuf_tensor(f"y1c{b}", [128, CH], bf16)) for b in range(2)]
        mk0 = ec(nc.sbuf_tensor("mk0", [128, CH], bf16))
        mk1 = ec(nc.sbuf_tensor("mk1", [128, CH], bf16))
        msum = [ec(nc.sbuf_tensor(f"msum{b}", [128, CH], bf16))
                for b in range(2)]
        wsb = ec(nc.sbuf_tensor("wsb", [128, CH], f32))
        junk2 = ec(nc.sbuf_tensor("junk2", [128, CH], f32))
        j2 = ec(nc.sbuf_tensor("j2", [128, CNT_COLS], f32))
        red = ec(nc.sbuf_tensor("red", [128, RED_COLS], f32))
        y0p = [ec(nc.psum_tensor(f"y0p{b}", [128, CH], f32)) for b in range(2)]
        y1p = [ec(nc.psum_tensor(f"y1p{b}", [128, CH], f32)) for b in range(2)]
        finq = ec(nc.psum_tensor("finq", [128, CH], f32))
        finu = ec(nc.psum_tensor("finu", [128, CH], f32))
        s_in = ec(nc.semaphore("s_in"))
        s_g = ec(nc.semaphore("s_g"))
        s_y = ec(nc.semaphore("s_y"))
        s_cp = ec(nc.semaphore("s_cp"))
        s_m = ec(nc.semaphore("s_m"))
        s_red = ec(nc.semaphore("s_red"))
        s_w = ec(nc.semaphore("s_w"))
        s_fin = ec(nc.semaphore("s_fin"))
        s_o = ec(nc.semaphore("s_o"))
        block = ec(nc.Block())
        NTOT = iters * N_CH

        @block.gpsimd
        def _(gp):
            gp.dma_start(idx_sb[:], idx[:]).then_inc(s_in, 16)
            gp.dma_start(tbl_sb[:], tbl[:]).then_inc(s_in, 16)
            gp.dma_start(cnt_sb[:], cnt[:]).then_inc(s_in, 16)
            gp.dma_start(asp_sb[:], asp[:]).then_inc(s_in, 16)
            gp.dma_start(oh0_sb[:], oh0[:]).then_inc(s_in, 16)
            gp.dma_start(oh1_sb[:], oh1[:]).then_inc(s_in, 16)
            gp.dma_start(m0_sb[:], m0[:]).then_inc(s_in, 16)
            gp.dma_start(m1_sb[:], m1[:]).then_inc(s_in, 16)
            gp.dma_start(sbo_sb[:], sbo[:]).then_inc(s_in, 16)
            gp.dma_start(stat_sb[:], stat[:]).then_inc(s_in, 16)
            gp.wait_ge(s_in, 160)
            for i in range(iters):
                # no waits: gathered content is iteration-invariant, so
                # benign buffer overwrite; a blocked gpsimd wait costs ~30us
                gp.ap_gather(
                    gath[i % 3][:], tbl_sb[:], idx_sb[:],
                    channels=128, num_elems=TBL_ROWS, d=DF, num_idxs=NIDX,
                ).then_inc(s_g, 1)

        @block.tensor
        def _(pe):
            pe.wait_ge(s_in, 160)

            def ymm(n):
                b = n % 2
                g = n % N_CH
                c0 = g * CH
                if n >= 2:  # Y[b] free once ACT copied chunk n-2
                    pe.wait_ge(s_cp, n - 1)
                pe.matmul(y0p[b][:], sbo_sb[:], oh0_sb[:, c0 : c0 + CH])
                pe.matmul(y1p[b][:], sbo_sb[:], oh1_sb[:, c0 : c0 + CH]) \
                    .then_inc(s_y, 1)

            def redmm(n):
                b = n % 2
                i = n // N_CH
                g = n % N_CH
                pe.wait_ge(s_m, n + 1)
                if g == 0 and i >= 1:  # FIN free once DVE finals iter i-1
                    pe.wait_ge(s_fin, i)
                sq = stat_sb[:, (2 * g) * 128 : (2 * g + 1) * 128]
                su = stat_sb[:, (2 * g + 1) * 128 : (2 * g + 2) * 128]
                pe.matmul(finq[:], sq, msum[b][:],
                          start=(g == 0), stop=(g == N_CH - 1))
                pe.matmul(finu[:], su, msum[b][:],
                          start=(g == 0), stop=(g == N_CH - 1)) \
                    .then_inc(s_red, 1)

            ymm(0)
            for n in range(NTOT):
                if n + 1 < NTOT:
                    ymm(n + 1)
                redmm(n)

        @block.scalar
        def _(sc):
            sc.wait_ge(s_in, 160)
            for i in range(iters):
                for g in range(N_CH):
                    n = i * N_CH + g
                    b = n % 2
                    sc.wait_ge(s_y, n + 1)
                    if n >= 2:  # y*c[b] free once DVE masked chunk n-2
                        sc.wait_ge(s_m, n - 1)
                    sc.copy(y0c[b][:], y0p[b][:])
                    sc.copy(y1c[b][:], y1p[b][:]).then_inc(s_cp, 1)
                # square of FINu on ACT (single PSUM operand per engine op)
                sc.wait_ge(s_red, (i + 1) * N_CH)
                sc.square(wsb[:], finu[:]).then_inc(s_w, 1)

        @block.vector
        def _(v):
            v.memset(red[:], 0.0)
            v.wait_ge(s_in, 160)
            for i in range(iters):
                v.scalar_tensor_tensor(
                    j2[:], cnt_sb[:], 1.0, asp_sb[:], alu.mult, alu.mult,
                    accum_out=red[:, 2:3],
                )
                # gather route
                ga = gath[i % 3]
                v.wait_ge(s_g, i + 1)
                v.tensor_tensor(sums[:], ga[:, 0:GP_B, :],
                                ga[:, GP_B:NIDX, :], alu.add)
                v.tensor_tensor(w2[:], sums[:, :, 1], sums[:, :, 1], alu.mult)
                v.scalar_tensor_tensor(
                    junk[:], sums[:, :, 0], 1.0, w2[:], alu.add, alu.mult,
                    accum_out=red[:, 0:1],
                )
                # PE route masks
                for g in range(N_CH):
                    n = i * N_CH + g
                    b = n % 2
                    c0 = g * CH
                    v.wait_ge(s_cp, n + 1)
                    if n >= 2:  # msum[b] free once PE reduced chunk n-2
                        v.wait_ge(s_red, n - 1)
                    v.tensor_tensor(mk0[:], y0c[b][:],
                                    m0_sb[:, c0 : c0 + CH], alu.mult)
                    v.tensor_tensor(mk1[:], y1c[b][:],
                                    m1_sb[:, c0 : c0 + CH], alu.mult)
                    v.tensor_tensor(msum[b][:], mk0[:], mk1[:], alu.add) \
                        .then_inc(s_m, 1)
                # finals
                v.wait_ge(s_w, i + 1)
                v.scalar_tensor_tensor(
                    junk2[:], finq[:], 1.0, wsb[:], alu.add, alu.mult,
                    accum_out=red[:, 4:5],
                ).then_inc(s_fin, 1)

        @block.sync
        def _(s):
            s.wait_ge(s_fin, iters)
            s.dma_start(out[:], red[:]).then_inc(s_o, 16)
            s.wait_ge(s_o, 16)

    nc.finalize()
    return nc


def _get_program(iters=1):
    if iters not in _PROGS:
        _PROGS[iters] = _build_program(iters)
    return _PROGS[iters]


def _make_tables(meta_theta, m_ks, grads_v):
    g = np.asarray(grads_v, np.float64)
    c = np.asarray(meta_theta, np.float64)[None, :] - np.asarray(m_ks, np.float64)
    h = g * c
    t_k = h.sum(axis=1)
    a_f = 0.5 * np.log(g) - 0.5 * g * c * c
    hp = h - 0.5 * t_k[:, None]
    ap = a_f + t_k[:, None] * h - 0.5 * (t_k * t_k)[:, None]
    q = -0.5 * g

    tbl = np.empty((TBL_ROWS, DF), np.float16)
    tbl[:, 0] = q.ravel().astype(np.float16)
    tbl[:, 1] = hp.ravel().astype(np.float16)

    asp = np.zeros(128 * CNT_COLS, np.float64)
    asp[:TBL_ROWS] = ap.ravel()
    asp = np.ascontiguousarray(asp.reshape(CNT_COLS, 128).T.astype(np.float32))

    bf = ml_dtypes.bfloat16
    sbo = np.zeros((128, 128), bf)
    sbo[:, 0:64] = q.ravel()[: 64 * 128].reshape(64, 128).T.astype(bf)
    sbo[:, 64:128] = hp.ravel()[: 64 * 128].reshape(64, 128).T.astype(bf)

    stat = np.zeros((128, 2 * N_CH * 128), bf)
    for g_ in range(N_CH):
        band = slice(16 * g_, 16 * (g_ + 1))
        sq = np.zeros((128, 128), np.float32)
        sq[0:64, band] = 1.0
        su = np.zeros((128, 128), np.float32)
        su[64:128, band] = 1.0
        stat[:, (2 * g_) * 128 : (2 * g_ + 1) * 128] = sq.astype(bf)
        stat[:, (2 * g_ + 1) * 128 : (2 * g_ + 2) * 128] = su.astype(bf)
    return tbl, asp, sbo, stat


def _device_inputs(meta_theta, m_ks, grads_v, perms):
    tbl, asp, sbo, stat = _make_tables(meta_theta, m_ks, grads_v)
    tbl_rep = np.ascontiguousarray(np.broadcast_to(tbl[None], (128, TBL_ROWS, DF)))
    bf = ml_dtypes.bfloat16

    perms01 = np.ascontiguousarray(np.asarray(perms)[:, :, :, :2])
    kvec = np.tile(np.arange(K, dtype=np.int64), TRIPLES // K)

    in_maps = []
    for core in range(N_CORES):
        sl = perms01[core * A_PER_CORE : (core + 1) * A_PER_CORE]
        sl = sl.reshape(TRIPLES, 2).astype(np.int64)
        comb = kvec[:, None] * D1 + sl                    # (4096, 2)

        cntv = np.bincount(comb.ravel(), minlength=128 * CNT_COLS)
        cnt = np.ascontiguousarray(
            cntv.reshape(CNT_COLS, 128).T.astype(np.float32)
        )

        # route: fringe pairs (touching rows >= 8192) must use the gather
        q_all = comb // 128
        elig = (q_all[:, 0] < 64) & (q_all[:, 1] < 64)
        pe_sel = np.where(elig)[0]
        assert len(pe_sel) >= PE_PAIRS, len(pe_sel)
        pe_sel = pe_sel[:PE_PAIRS]
        g_sel = np.setdiff1d(np.arange(TRIPLES), pe_sel)
        assert len(g_sel) == G_PAIRS

        # gather route packing
        cg = comb[g_sel].reshape(8, GP_B, 2)
        L = np.empty((8, NIDX), np.int16)
        L[:, :GP_B] = cg[:, :, 0]
        L[:, GP_B:] = cg[:, :, 1]
        idx128 = L.reshape(8, IDX_COLS, 16).transpose(0, 2, 1)
        idx128 = np.ascontiguousarray(idx128.reshape(128, IDX_COLS))

        # PE route packing
        cpe = comb[pe_sel]
        r = (cpe % 128).astype(np.int64)
        q = (cpe // 128).astype(np.int64)
        oh0 = np.zeros((128, PE_PAIRS), bf)
        oh1 = np.zeros((128, PE_PAIRS), bf)
        m0 = np.zeros((128, PE_PAIRS), bf)
        m1 = np.zeros((128, PE_PAIRS), bf)
        t_ar = np.arange(PE_PAIRS)
        oh0[r[:, 0], t_ar] = 1
        oh1[r[:, 1], t_ar] = 1
        m0[q[:, 0], t_ar] = 1
        m0[64 + q[:, 0], t_ar] = 1
        m1[q[:, 1], t_ar] = 1
        m1[64 + q[:, 1], t_ar] = 1

        in_maps.append({
            "tbl": tbl_rep, "idx": idx128, "cnt": cnt, "asp": asp,
            "oh0": oh0, "oh1": oh1, "m0": m0, "m1": m1,
            "sbo": sbo, "stat": stat,
        })
    return in_maps


def _finalize(partials, meta_theta, alpha):
    p = np.asarray(partials, np.float64)
    total = float(
        p[:, ::16, 0].sum() + p[:, ::16, 4].sum() + p[:, :, 2].sum()
    )
    sum_lp = total - LOG2PI * (N_CORES * TRIPLES)
    loss_pred = sum_lp / (P * M_COND * K)
    mt = np.asarray(meta_theta, np.float64)
    a = float(alpha)
    lp_prior = -0.5 * (D1 * LOG2PI + D1 * np.log(a) + float(mt @ mt) / a)
    loss = (1.0 - 1.0 / K) * lp_prior + loss_pred
    return np.float32(-loss)


def run_device(in_maps, iters=1, **kwargs):
    nc = _get_program(iters)
    return run_bass_kernel_spmd(nc, in_maps, list(range(N_CORES)), **kwargs)


def kernel(meta_theta, m_ks, grads_v, perms, alpha):
    in_maps = _device_inputs(meta_theta, m_ks, grads_v, perms)
    last_err = None
    for _ in range(3):
        try:
            res = run_device(in_maps)
            break
        except Exception as e:  # noqa: BLE001
            last_err = e
    else:
        raise last_err
    partials = np.stack([r["out"] for r in res.results])
    return _finalize(partials, meta_theta, alpha)
